# revision 1
# baseline (speedup 1.0000x reference)
"""Trainium2 Bass kernel for nn_AttentionBranch: conv->relu->maxpool->conv->relu
followed by per-location rank-1 Gram outer products (100, 1024, 1024).

Sharding: the 100-location Gram axis is split across 8 NeuronCores
(13/12 locations per core). The conv backbone is replicated (conv1) /
channel-sliced to each core's needed 136-channel window (conv2), so no
collectives are required. The row-major .view(100, 1024) of the conv2
output is realised through a tiny DRAM scratch roundtrip.

Numerics: conv1 runs in fp32r (TensorE full-rate, ~2e-4), conv2 in bf16
(~2e-3, well inside the 2e-2 gate), and the Gram products exactly in
fp32 on VectorE/ScalarE (tensor_scalar against a PE-broadcast row tile).

Perf structure: input/weight loads are spread over the SP/ACT/GPSIMD DMA
queues with conv1's first ci-chunk prioritized. The per-core flat-view
shift (delta 0 vs 12) is folded into the Gram-stage PE matmuls via
per-core select weights, so no vector-engine select pass is needed.
Output staging interleaves 4 gram rows per SBUF partition so each 2 MiB
store is one contiguous 16 KiB run per partition, with the SP and ACT
descriptor queues alternating by row parity.
"""
import os
import numpy as np

# per-core location starts (each core computes 13 consecutive locations;
# odd cores' 13th overlaps the next core, core 7's 13th is garbage)
_LO = [0, 12, 25, 37, 50, 62, 75, 87]
# conv2 channel-slice starts; delta_k = 1024*lo_k - 100*ch_lo_k is 0 (even k)
# or 12 (odd k)
_CH_LO = [0, 122, 256, 378, 512, 634, 768, 890]
_NSL = 136  # channels per conv2 slice (covers 12 + 13*1024 flat elements)

_CACHE = {}


def _build_nc():
    from concourse import bacc, tile, mybir

    f32 = mybir.dt.float32
    f32r = mybir.dt.float32r
    f16 = mybir.dt.float16
    bf16 = mybir.dt.bfloat16
    AF = mybir.ActivationFunctionType

    nc = bacc.Bacc("TRN2", target_bir_lowering=False, debug=False)

    inp_d = nc.dram_tensor("inp", [128, 4, 27, 25], f32r, kind="ExternalInput")
    w1_d = nc.dram_tensor("w1t", [128, 4, 9, 64], f32r, kind="ExternalInput")
    b1_d = nc.dram_tensor("b1t", [64, 1], f32, kind="ExternalInput")
    psh_d = nc.dram_tensor("pshard", [64, 144], bf16)
    pfull_d = nc.dram_tensor("pfull", [512, 144], bf16, addr_space="Shared")
    w2_d = nc.dram_tensor("w2t", [128, 4, 9, _NSL], bf16, kind="ExternalInput")
    b2_d = nc.dram_tensor("b2t", [128, 2], f32, kind="ExternalInput")
    selw_d = nc.dram_tensor("selw", [2, 128], f32r, kind="ExternalInput")
    selid_d = nc.dram_tensor("selid", [26, 16], f32, kind="ExternalInput")
    gp_d = nc.dram_tensor("gpart", [13, 512, 1024], f16, kind="ExternalOutput")
    aux_d = nc.dram_tensor("gaux", [13, 512, 512], f16, kind="ExternalOutput")
    scr_d = nc.dram_tensor("scratch", [137, 100], f32)

    with tile.TileContext(nc) as tc:
        with tc.tile_pool(name="consts", bufs=1) as cp, \
             tc.tile_pool(name="work", bufs=1) as wp:

            w2sb = cp.tile([128, 4, 9, _NSL], bf16)
            b1sb = cp.tile([64, 1], f32)
            b2sb = cp.tile([128, 2], f32)
            selwsb = cp.tile([2, 128], f32r)
            selidsb = cp.tile([26, 16], f32)

            # ---- conv1 inputs first, spread across DMA queues ----
            convp = tc.alloc_tile_pool(name="convp", bufs=1)
            ps1 = tc.alloc_tile_pool(name="ps1", bufs=1, space="PSUM")
            insb = convp.tile([128, 4, 27, 25], f32r)
            w1sb = convp.tile([128, 4, 9, 64], f32r)
            for c in range(4):
                nc.gpsimd.dma_start(out=insb[:, c], in_=inp_d.ap()[:, c])
                nc.sync.dma_start(out=w1sb[:, c], in_=w1_d.ap()[:, c])

            # consts on the scalar queue (needed only after conv1)
            nc.scalar.dma_start(out=w2sb[:], in_=w2_d.ap())
            nc.scalar.dma_start(out=b1sb[:], in_=b1_d.ap())
            nc.scalar.dma_start(out=b2sb[:], in_=b2_d.ap())
            nc.scalar.dma_start(out=selwsb[:], in_=selw_d.ap())
            nc.scalar.dma_start(out=selidsb[:], in_=selid_d.ap())

            # ---- conv1 (sharded): this core computes output channels
            # [64k, 64k+64) of (512,23,23); the pooled bf16 shards are
            # allgathered so every core gets the full conv2 input. ----
            c1sb = wp.tile([128, 24, 24], f32)
            nc.vector.memset(c1sb[0:64, 23:24, :], 0.0)
            nc.vector.memset(c1sb[0:64, :, 23:24], 0.0)
            c1groups = [(0, 12), (12, 11)]
            c1ps = [ps1.tile([128, 300], f32, tag=f"c1p{gi}",
                             name=f"c1ps{gi}") for gi in range(2)]
            for c in range(4):
                flat_c = insb[:, c].rearrange("p a b -> p (a b)")
                for gi, (r0, nr) in enumerate(c1groups):
                    for t in range(9):
                        dy, dx = t // 3, t % 3
                        s0 = (r0 + dy) * 25 + dx
                        nc.tensor.matmul(
                            c1ps[gi][0:64, :],
                            w1sb[:, c, t, :],
                            flat_c[:, s0:s0 + 300],
                            start=(c == 0 and t == 0),
                            stop=(c == 3 and t == 8),
                        )
            for gi, (r0, nr) in enumerate(c1groups):
                nc.scalar.activation(
                    out=c1sb[0:64, r0:r0 + nr, 0:23],
                    in_=c1ps[gi][0:64, 0:300].rearrange("p (a b) -> p a b", b=25)[:, 0:nr, 0:23],
                    func=AF.Relu,
                    bias=b1sb[:, 0:1],
                )

            # ---- maxpool 2x2 ceil on the shard -> (64,12,12) bf16 ----
            colmax = wp.tile([128, 24, 12], f32)
            cpair = c1sb[:].rearrange("p r (w two) -> p r w two", two=2)
            nc.vector.tensor_max(colmax[0:64], cpair[0:64, :, :, 0],
                                 cpair[0:64, :, :, 1])
            pooledsh = wp.tile([128, 12, 12], bf16)
            rpair = colmax[:].rearrange("p (r two) w -> p r two w", two=2)
            nc.vector.tensor_max(pooledsh[0:64], rpair[0:64, :, 0, :],
                                 rpair[0:64, :, 1, :])
            nc.sync.dma_start(
                out=psh_d.ap(),
                in_=pooledsh[0:64].rearrange("p h w -> p (h w)"))
            nc.gpsimd.collective_compute(
                "AllGather",
                mybir.AluOpType.bypass,
                replica_groups=[list(range(8))],
                ins=[psh_d.ap()],
                outs=[pfull_d.ap()],
            )
            pooled = wp.tile([128, 4, 12, 12], bf16)
            pfv = pfull_d.ap().rearrange("(c p) f -> c p f", p=128)
            for c in range(4):
                eng = nc.sync if c % 2 == 0 else nc.scalar
                eng.dma_start(
                    out=pooled[:, c].rearrange("p h w -> p (h w)"),
                    in_=pfv[c])

            # ---- conv2 slice: 136 output channels, bf16 (TensorE full rate) ----
            ps1.release()
            ps2 = tc.alloc_tile_pool(name="ps2", bufs=2, space="PSUM")
            c2sb = wp.tile([128, 2, 100], f32)
            for m, (mo, mw) in enumerate([(0, 128), (128, 8)]):
                ps = ps2.tile([128, 100], f32, tag="c2p")
                for c in range(4):
                    for t in range(9):
                        dy, dx = t // 3, t % 3
                        nc.tensor.matmul(
                            ps[0:mw, :],
                            w2sb[:, c, t, mo:mo + mw],
                            pooled[:, c, dy:dy + 10, dx:dx + 10],
                            start=(c == 0 and t == 0),
                            stop=(c == 3 and t == 8),
                        )
                nc.scalar.activation(
                    out=c2sb[0:mw, m, :],
                    in_=ps[0:mw, :],
                    func=AF.Relu,
                    bias=b2sb[0:mw, m:m + 1],
                )
                # flat view via DRAM scratch: store each chunk as it lands
                if m == 0:
                    nc.sync.dma_start(out=scr_d.ap()[0:128, :], in_=c2sb[:, 0, :])
                else:
                    nc.gpsimd.dma_start(out=scr_d.ap()[128:136, :],
                                        in_=c2sb[0:8, 1, :])

            # T2: T0 rows (flat offset 0) on partitions 0-12, T12 rows
            # (flat offset 12) on partitions 13-25. The delta select is
            # folded into the Gram-stage matmuls via selw/selid.
            flat = scr_d.ap().rearrange("a b -> (a b)")
            T2 = wp.tile([26, 1024], f32)
            t2row = wp.tile([2, 13312], f32r)
            # head loads (rows 0-11) depend only on the m=0 chunk of scratch;
            # keep them on the fast HWDGE queues (sync/scalar)
            nc.sync.dma_start(out=t2row[0:1, 0:12288],
                              in_=flat[0:12288].bitcast(f32r))
            nc.scalar.dma_start(out=t2row[1:2, 0:12288],
                                in_=flat[88:12376].bitcast(f32r))
            nc.sync.dma_start(
                out=T2[0:12, :],
                in_=flat[0:12288].rearrange("(p i) -> p i", i=1024))
            nc.scalar.dma_start(
                out=T2[13:25, :],
                in_=flat[88:12376].rearrange("(p i) -> p i", i=1024))
            # tails (row 12) additionally need the m=1 chunk
            nc.sync.dma_start(out=t2row[0:1, 12288:13312],
                              in_=flat[12288:13312].bitcast(f32r))
            nc.scalar.dma_start(out=t2row[1:2, 12288:13312],
                                in_=flat[12376:13400].bitcast(f32r))
            nc.gpsimd.dma_start(
                out=T2[12:13, :],
                in_=flat[12288:13312].rearrange("(p i) -> p i", i=1024))
            nc.gpsimd.dma_start(
                out=T2[25:26, :],
                in_=flat[12376:13400].rearrange("(p i) -> p i", i=1024))

            ps2.release()
            convp.release()

            vp = tc.alloc_tile_pool(name="bcast", bufs=3)
            sp = tc.alloc_tile_pool(name="stage", bufs=6)
            psT = tc.alloc_tile_pool(name="psT", bufs=2, space="PSUM")
            psB = tc.alloc_tile_pool(name="psB", bufs=3, space="PSUM")

            tcol = wp.tile([128, 8, 16], f32)

            # tcol[p, 4u+x, l] = v_l[512u + 4p + x]  (4-row interleave),
            # via one K=26 matmul per (u,x) against the selid block-diagonal
            # (s0*I on partitions 0-12, s1*I on 13-25) -- select for free.
            for u in range(2):
                lhs4 = T2[:, 512 * u:512 * (u + 1)].rearrange(
                    "l (m four) -> l four m", four=4)
                for x in range(4):
                    pst = psT.tile([128, 16], f32, tag="tc")
                    nc.tensor.matmul(
                        pst[:, 0:13], lhs4[:, x, :], selidsb[0:26, 0:13],
                        start=True, stop=True,
                    )
                    nc.vector.tensor_copy(tcol[:, 4 * u + x, 0:13],
                                          pst[:, 0:13])

            # rank parity drives which edge half-unit this core stores
            _g8 = [list(range(8))]
            par_s = nc.sync.cc_rank(replica_groups=_g8) % 2
            par_a = nc.scalar.cc_rank(replica_groups=_g8) % 2

            # ---- Gram outer products, exact fp32 on DVE/ACT ----
            for li in range(13):
                # broadcast row li to all 128 partitions; K=2 matmul applies
                # the T0/T12 select via selw. fp32r: full rate at free>=256.
                bp = psB.tile([128, 1024], f32, tag="bc")
                nc.tensor.matmul(bp[:, 0:512], selwsb[:],
                                 t2row[:, 1024 * li:1024 * li + 512],
                                 start=True, stop=True)
                nc.tensor.matmul(bp[:, 512:1024], selwsb[:],
                                 t2row[:, 1024 * li + 512:1024 * (li + 1)],
                                 start=True, stop=True)
                bc = vp.tile([128, 1024], f32, tag="bcs")
                nc.vector.tensor_copy(bc[:, 0:512], bp[:, 0:512])
                nc.scalar.activation(bc[:, 512:1024], bp[:, 512:1024],
                                     func=AF.Copy)
                # gram[l] is symmetric: store rows 0-511 full width and
                # block (1,1) compactly to gaux; block (1,0) is the host-side
                # transpose of block (0,1).
                for u in range(2):
                    if u == 0:
                        st = sp.tile([128, 4096], f16, tag="st")
                        wj, base = 1024, 0
                    else:
                        st = sp.tile([128, 2048], f16, tag="st1")
                        wj, base = 512, 512
                    for x in range(4):
                        col = tcol[:, 4 * u + x, li:li + 1]
                        dve = (x % 2 == 0) or (u == 1 and x == 3)
                        if dve:
                            nc.vector.tensor_scalar_mul(
                                st[:, x * wj:(x + 1) * wj],
                                bc[:, base:1024], col)
                        else:
                            nc.scalar.activation(
                                st[:, x * wj:(x + 1) * wj], bc[:, base:1024],
                                func=AF.Copy, scale=col)
                    if u == 0:
                        dst = gp_d.ap()[li, :, :].rearrange(
                            "(q four) f -> q (four f)", four=4)
                    else:
                        dst = aux_d.ap()[li, :, :].rearrange(
                            "(q four) f -> q (four f)", four=4)
                    if (u + li) % 2 == 0:
                        # (0,0) belongs to even cores only
                        cond = (1 - par_s) if (li == 0 and u == 0) else None
                        nc.sync.dma_start(out=dst, in_=st[:], cond=cond)
                    else:
                        # (12,1) belongs to odd cores only
                        cond = par_a if (li == 12 and u == 1) else None
                        nc.scalar.dma_start(out=dst, in_=st[:], cond=cond)
            psB.release()
            psT.release()
            sp.release()
            vp.release()

    nc.compile()
    return nc


def _get_nc():
    if "nc" not in _CACHE:
        _CACHE["nc"] = _build_nc()
    return _CACHE["nc"]


def _host_prep(input, w1, b1, w2, b2):
    import ml_dtypes

    x = np.asarray(input, np.float32).reshape(512, 25, 25)
    w1 = np.asarray(w1, np.float32)
    w2 = np.asarray(w2, np.float32)
    b1 = np.asarray(b1, np.float32)
    b2 = np.asarray(b2, np.float32)

    inp = np.zeros((4, 128, 27, 25), np.float32)
    inp[:, :, :25, :] = x.reshape(4, 128, 25, 25)
    inp = np.ascontiguousarray(inp.transpose(1, 0, 2, 3))

    w1t = w1.reshape(512, 512, 9).transpose(1, 2, 0)          # [ci, 9, co]
    w1t = np.ascontiguousarray(
        w1t.reshape(4, 128, 9, 512).transpose(1, 0, 2, 3))    # [128,4,9,512]

    common = {"inp": inp}
    in_maps = []
    for k in range(8):
        ch = _CH_LO[k]
        nval = min(1024, ch + _NSL) - ch
        wsl = np.zeros((_NSL, 512, 9), np.float32)
        wsl[:nval] = w2.reshape(1024, 512, 9)[ch:ch + nval]
        w2t = wsl.transpose(1, 2, 0)                           # [512,9,136]
        w2t = np.ascontiguousarray(
            w2t.reshape(4, 128, 9, _NSL).transpose(1, 0, 2, 3)).astype(
                ml_dtypes.bfloat16)
        bsl = np.zeros(256, np.float32)
        bsl[:nval] = b2[ch:ch + nval]
        b2t = np.ascontiguousarray(bsl.reshape(2, 128).T)
        s0 = 1.0 if (1024 * _LO[k] - 100 * ch) == 0 else 0.0
        selw = np.zeros((2, 128), np.float32)
        selw[0, :] = s0
        selw[1, :] = 1.0 - s0
        selid = np.zeros((26, 16), np.float32)
        selid[0:13, 0:13] = s0 * np.eye(13, dtype=np.float32)
        selid[13:26, 0:13] = (1.0 - s0) * np.eye(13, dtype=np.float32)
        w1sl = np.ascontiguousarray(w1t[:, :, :, 64 * k:64 * k + 64])
        b1sl = np.ascontiguousarray(b1[64 * k:64 * k + 64].reshape(64, 1))
        in_maps.append({**common, "w1t": w1sl, "b1t": b1sl,
                        "w2t": w2t, "b2t": b2t,
                        "selw": selw, "selid": selid})
    return in_maps


def kernel(input, w1, b1, w2, b2):
    from concourse import bass_utils

    nc = _get_nc()
    in_maps = _host_prep(input, w1, b1, w2, b2)

    prof_dir = os.environ.get("GRAM_KERNEL_PROFILE_DIR")
    if prof_dir:
        from trn_agent_boot.trn_boot import _ntff_profile_via_ctypes
        hook = _ntff_profile_via_ctypes('/opt/axon/libaxon_pjrt.so')
        with hook(prof_dir, [0]):
            res = bass_utils.run_bass_kernel_spmd(
                nc, in_maps, core_ids=list(range(8)))
    else:
        res = bass_utils.run_bass_kernel_spmd(
            nc, in_maps, core_ids=list(range(8)))

    out = np.empty((100, 1024, 1024), np.float32)
    for k in range(8):
        g = res.results[k]["gpart"]
        a = res.results[k]["gaux"]
        lo = _LO[k]
        if k % 2 == 0:
            out[lo:lo + 13, 0:512, :] = g
            out[lo:lo + 12, 512:1024, 512:1024] = a[0:12]
        else:
            out[lo + 1:lo + 13, 0:512, :] = g[1:13]
            out[lo:lo + 13, 512:1024, 512:1024] = a
    # block (1,0) is the transpose of the stored block (0,1)
    out[:, 512:1024, 0:512] = out[:, 0:512, 512:1024].transpose(0, 2, 1)
    return out



# revision 2
# speedup vs baseline: 1.0387x; 1.0387x over previous
"""Trainium2 Bass kernel for nn_AttentionBranch: conv->relu->maxpool->conv->relu
followed by per-location rank-1 Gram outer products (100, 1024, 1024).

Sharding: the 100-location Gram axis is split across 8 NeuronCores
(13/12 locations per core). The conv1 backbone is fully REPLICATED on
every core (bf16) so no collective is needed at all -- the previous
AllGather cost ~50us of rendezvous latency. conv2 is channel-sliced to
each core's needed 136-channel window. The row-major .view(100, 1024)
of the conv2 output is realised through a small DRAM scratch roundtrip.

Output: per location only the upper "staircase" of 36 of the 64
128x128 blocks of the symmetric Gram matrix is computed and stored
(row-block r keeps columns [128r, 1024)), one contiguous 9216B/partition
store per location. The host reconstructs the lower blocks by
transposition. This cuts HBM store traffic by 25% vs storing the upper
512-row half plus the (1,1) block.

Numerics: conv1/conv2 run in bf16 (PE full rate), the Gram products in
fp32 on DVE/ACT (tensor_scalar against a PE-broadcast row tile read
directly from PSUM), stored as f16.
"""
import os
import numpy as np

# per-core location starts (each core computes 13 consecutive locations;
# odd cores' first location duplicates the previous core's last)
_LO = [0, 12, 25, 37, 50, 62, 75, 87]
# conv2 channel-slice starts; delta_k = 1024*lo_k - 100*ch_lo_k is 0 (even k)
# or 88 (odd k)
_CH_LO = [0, 122, 256, 378, 512, 634, 768, 890]
_NSL = 136  # channels per conv2 slice (covers 88 + 13*1024 flat elements)
# staircase column offsets: block r (width 1024-128r) starts at _OFF[r]
_OFF = [0, 1024, 1920, 2688, 3328, 3840, 4224, 4480]

_CACHE = {}


def _build_nc():
    from concourse import bacc, tile, mybir

    f32 = mybir.dt.float32
    f32r = mybir.dt.float32r
    f16 = mybir.dt.float16
    bf16 = mybir.dt.bfloat16
    AF = mybir.ActivationFunctionType

    nc = bacc.Bacc("TRN2", target_bir_lowering=False, debug=False)

    inp_d = nc.dram_tensor("inp", [128, 4, 27, 25], bf16, kind="ExternalInput")
    w1_d = nc.dram_tensor("w1t", [128, 4, 9, 512], bf16, kind="ExternalInput")
    b1_d = nc.dram_tensor("b1t", [128, 4], f32, kind="ExternalInput")
    w2_d = nc.dram_tensor("w2t", [128, 4, 9, _NSL], bf16, kind="ExternalInput")
    b2_d = nc.dram_tensor("b2t", [128, 2], f32, kind="ExternalInput")
    selw_d = nc.dram_tensor("selw", [2, 128], f32r, kind="ExternalInput")
    selid_d = nc.dram_tensor("selid", [26, 16], f32, kind="ExternalInput")
    g_d = nc.dram_tensor("gstair", [13, 128, 4608], f16, kind="ExternalOutput")
    scr_d = nc.dram_tensor("scratch", [137, 100], f32)

    with tile.TileContext(nc) as tc:
        with tc.tile_pool(name="consts", bufs=1) as cp, \
             tc.tile_pool(name="work", bufs=1) as wp:

            w2sb = cp.tile([128, 4, 9, _NSL], bf16)
            b1sb = cp.tile([128, 4], f32)
            b2sb = cp.tile([128, 2], f32)
            selwsb = cp.tile([2, 128], f32r)
            selidsb = cp.tile([26, 16], f32)

            convp = tc.alloc_tile_pool(name="convp", bufs=1)
            ps1 = tc.alloc_tile_pool(name="ps1", bufs=1, space="PSUM")
            insb = convp.tile([128, 4, 27, 25], bf16)
            w1sb = [convp.tile([128, 9, 512], bf16, name=f"w1c{c}")
                    for c in range(4)]

            # critical loads first, alternating the two HWDGE queues
            nc.sync.dma_start(out=insb[:], in_=inp_d.ap())
            nc.scalar.dma_start(out=w1sb[0][:], in_=w1_d.ap()[:, 0])
            nc.sync.dma_start(out=w1sb[1][:], in_=w1_d.ap()[:, 1])
            nc.scalar.dma_start(out=w1sb[2][:], in_=w1_d.ap()[:, 2])
            nc.sync.dma_start(out=w1sb[3][:], in_=w1_d.ap()[:, 3])
            nc.scalar.dma_start(out=w2sb[:], in_=w2_d.ap())
            nc.scalar.dma_start(out=b1sb[:], in_=b1_d.ap())
            nc.scalar.dma_start(out=b2sb[:], in_=b2_d.ap())
            nc.scalar.dma_start(out=selwsb[:], in_=selw_d.ap())
            nc.scalar.dma_start(out=selidsb[:], in_=selid_d.ap())

            # ---- conv1 FULL (512 channels), bf16, c-outer so matmuls can
            # start as soon as the first ci-chunk of weights lands ----
            GB = [(0, 12, 300), (12, 11, 275)]  # (row0, nrows, N)
            c1ps = [[ps1.tile([128, GB[g][2]], f32, name=f"c1ps{m}_{g}")
                     for g in range(2)] for m in range(4)]
            c1sb = wp.tile([128, 4, 24, 24], f32)
            nc.vector.memset(c1sb[:, :, 23:24, :], 0.0)
            nc.vector.memset(c1sb[:, :, :, 23:24], 0.0)

            for c in range(4):
                flat_c = insb[:, c].rearrange("p a b -> p (a b)")
                for m in range(4):
                    for t in range(9):
                        dy, dx = t // 3, t % 3
                        for g, (r0, nr, N) in enumerate(GB):
                            s0 = (r0 + dy) * 25 + dx
                            nc.tensor.matmul(
                                c1ps[m][g][:, :],
                                w1sb[c][:, t, 128 * m:128 * m + 128],
                                flat_c[:, s0:s0 + N],
                                start=(c == 0 and t == 0),
                                stop=(c == 3 and t == 8),
                            )

            # relu+bias then maxpool 2x2 ceil per m-chunk -> (512,12,12) bf16
            colmax = wp.tile([128, 4, 24, 12], f32)
            pooled = wp.tile([128, 4, 12, 12], bf16)
            for m in range(4):
                for g, (r0, nr, N) in enumerate(GB):
                    nc.scalar.activation(
                        out=c1sb[:, m, r0:r0 + nr, 0:23],
                        in_=c1ps[m][g][:, :].rearrange(
                            "p (a b) -> p a b", b=25)[:, 0:nr, 0:23],
                        func=AF.Relu,
                        bias=b1sb[:, m:m + 1],
                    )
                cpair = c1sb[:, m].rearrange("p r (w two) -> p r w two", two=2)
                nc.vector.tensor_max(colmax[:, m], cpair[:, :, :, 0],
                                     cpair[:, :, :, 1])
                rpair = colmax[:, m].rearrange("p (r two) w -> p r two w",
                                               two=2)
                nc.vector.tensor_max(pooled[:, m], rpair[:, :, 0, :],
                                     rpair[:, :, 1, :])

            # ---- conv2 slice: 136 output channels, bf16 ----
            ps1.release()
            ps2 = tc.alloc_tile_pool(name="ps2", bufs=2, space="PSUM")
            c2sb = wp.tile([128, 2, 100], f32)
            for mi, (mo, mw) in enumerate([(0, 128), (128, 8)]):
                ps = ps2.tile([128, 100], f32, tag="c2p")
                for c in range(4):
                    for t in range(9):
                        dy, dx = t // 3, t % 3
                        nc.tensor.matmul(
                            ps[0:mw, :],
                            w2sb[:, c, t, mo:mo + mw],
                            pooled[:, c, dy:dy + 10, dx:dx + 10],
                            start=(c == 0 and t == 0),
                            stop=(c == 3 and t == 8),
                        )
                nc.scalar.activation(
                    out=c2sb[0:mw, mi, :],
                    in_=ps[0:mw, :],
                    func=AF.Relu,
                    bias=b2sb[0:mw, mi:mi + 1],
                )
                # flat view via DRAM scratch: store each chunk as it lands
                if mi == 0:
                    nc.sync.dma_start(out=scr_d.ap()[0:128, :],
                                      in_=c2sb[:, 0, :])
                else:
                    nc.scalar.dma_start(out=scr_d.ap()[128:136, :],
                                        in_=c2sb[0:8, 1, :])

            # flat .view(13,1024) rows: T2 holds both delta variants for the
            # column factors; t2row holds them as 2 long rows for the PE
            # row-broadcast. Head parts depend only on the m=0 scratch chunk.
            flat = scr_d.ap().rearrange("a b -> (a b)")
            T2 = wp.tile([26, 1024], f32)
            t2row = wp.tile([2, 13312], f32r)
            nc.sync.dma_start(out=t2row[0:1, 0:12288],
                              in_=flat[0:12288].bitcast(f32r))
            nc.scalar.dma_start(out=t2row[1:2, 0:12288],
                                in_=flat[88:12376].bitcast(f32r))
            nc.sync.dma_start(
                out=T2[0:12, :],
                in_=flat[0:12288].rearrange("(p i) -> p i", i=1024))
            nc.scalar.dma_start(
                out=T2[13:25, :],
                in_=flat[88:12376].rearrange("(p i) -> p i", i=1024))
            nc.sync.dma_start(out=t2row[0:1, 12288:13312],
                              in_=flat[12288:13312].bitcast(f32r))
            nc.scalar.dma_start(out=t2row[1:2, 12288:13312],
                                in_=flat[12376:13400].bitcast(f32r))
            nc.sync.dma_start(
                out=T2[12:13, :],
                in_=flat[12288:13312].rearrange("(p i) -> p i", i=1024))
            nc.scalar.dma_start(
                out=T2[25:26, :],
                in_=flat[12376:13400].rearrange("(p i) -> p i", i=1024))

            ps2.release()
            convp.release()

            # column factors: ccol[p, r, l] = v_l[128r + p], selecting the
            # delta-0/delta-88 variant via the selid block-diagonal
            psC = tc.alloc_tile_pool(name="psC", bufs=1, space="PSUM")
            pc = psC.tile([128, 8, 16], f32)
            ccol = wp.tile([128, 8, 16], f32)
            for r in range(8):
                nc.tensor.matmul(pc[:, r, 0:13], T2[:, 128 * r:128 * r + 128],
                                 selidsb[0:26, 0:13], start=True, stop=True)
            nc.vector.tensor_copy(ccol[:, :, 0:13], pc[:, :, 0:13])

            psB = tc.alloc_tile_pool(name="psB", bufs=3, space="PSUM")
            sp = tc.alloc_tile_pool(name="stage", bufs=5)

            # ---- Gram staircase, exact fp32 on DVE/ACT from PSUM ----
            ACT_R = (0, 3, 7)  # ACT gets 1024+640+128, DVE the other 2816
            for li in range(13):
                bp = psB.tile([128, 1024], f32, tag="bc")
                nc.tensor.matmul(bp[:, 0:512], selwsb[:],
                                 t2row[:, 1024 * li:1024 * li + 512],
                                 start=True, stop=True)
                nc.tensor.matmul(bp[:, 512:1024], selwsb[:],
                                 t2row[:, 1024 * li + 512:1024 * (li + 1)],
                                 start=True, stop=True)
                st = sp.tile([128, 4608], f16, tag="st")
                for r in range(8):
                    w = 1024 - 128 * r
                    src = bp[:, 128 * r:1024]
                    dst = st[:, _OFF[r]:_OFF[r] + w]
                    col = ccol[:, r, li:li + 1]
                    if r in ACT_R:
                        nc.scalar.activation(dst, src, func=AF.Copy,
                                             scale=col)
                    else:
                        nc.vector.tensor_scalar_mul(dst, src, col)
                eng = nc.sync if li % 2 == 0 else nc.scalar
                eng.dma_start(out=g_d.ap()[li], in_=st[:])
            psB.release()
            psC.release()
            sp.release()

    nc.compile()
    return nc


def _get_nc():
    if "nc" not in _CACHE:
        _CACHE["nc"] = _build_nc()
    return _CACHE["nc"]


def _host_prep(input, w1, b1, w2, b2):
    import ml_dtypes
    bf = ml_dtypes.bfloat16

    x = np.asarray(input, np.float32).reshape(512, 25, 25)
    w1 = np.asarray(w1, np.float32)
    w2 = np.asarray(w2, np.float32)
    b1 = np.asarray(b1, np.float32)
    b2 = np.asarray(b2, np.float32)

    inp = np.zeros((4, 128, 27, 25), np.float32)
    inp[:, :, :25, :] = x.reshape(4, 128, 25, 25)
    inp = np.ascontiguousarray(inp.transpose(1, 0, 2, 3)).astype(bf)

    w1t = w1.reshape(512, 512, 9).transpose(1, 2, 0)          # [ci, 9, co]
    w1t = np.ascontiguousarray(
        w1t.reshape(4, 128, 9, 512).transpose(1, 0, 2, 3)).astype(bf)
    b1t = np.ascontiguousarray(b1.reshape(4, 128).T)          # [128, 4]

    common = {"inp": inp, "w1t": w1t, "b1t": b1t}
    in_maps = []
    for k in range(8):
        ch = _CH_LO[k]
        nval = min(1024, ch + _NSL) - ch
        wsl = np.zeros((_NSL, 512, 9), np.float32)
        wsl[:nval] = w2.reshape(1024, 512, 9)[ch:ch + nval]
        w2t = wsl.transpose(1, 2, 0)                           # [512,9,136]
        w2t = np.ascontiguousarray(
            w2t.reshape(4, 128, 9, _NSL).transpose(1, 0, 2, 3)).astype(bf)
        bsl = np.zeros(256, np.float32)
        bsl[:nval] = b2[ch:ch + nval]
        b2t = np.ascontiguousarray(bsl.reshape(2, 128).T)
        s0 = 1.0 if (1024 * _LO[k] - 100 * ch) == 0 else 0.0
        selw = np.zeros((2, 128), np.float32)
        selw[0, :] = s0
        selw[1, :] = 1.0 - s0
        selid = np.zeros((26, 16), np.float32)
        selid[0:13, 0:13] = s0 * np.eye(13, dtype=np.float32)
        selid[13:26, 0:13] = (1.0 - s0) * np.eye(13, dtype=np.float32)
        in_maps.append({**common, "w2t": w2t, "b2t": b2t,
                        "selw": selw, "selid": selid})
    return in_maps


def kernel(input, w1, b1, w2, b2):
    from concourse import bass_utils

    nc = _get_nc()
    in_maps = _host_prep(input, w1, b1, w2, b2)

    prof_dir = os.environ.get("GRAM_KERNEL_PROFILE_DIR")
    if prof_dir:
        from trn_agent_boot.trn_boot import _ntff_profile_via_ctypes
        hook = _ntff_profile_via_ctypes('/opt/axon/libaxon_pjrt.so')
        with hook(prof_dir, [0]):
            res = bass_utils.run_bass_kernel_spmd(
                nc, in_maps, core_ids=list(range(8)))
    else:
        res = bass_utils.run_bass_kernel_spmd(
            nc, in_maps, core_ids=list(range(8)))

    out = np.empty((100, 1024, 1024), np.float32)
    for k in range(8):
        S = np.asarray(res.results[k]["gstair"])   # [13, 128, 4608] f16
        j0 = k % 2   # odd cores' first row duplicates previous core's last
        lo = _LO[k]
        for r in range(8):
            w = 1024 - 128 * r
            out[lo + j0:lo + 13, 128 * r:128 * r + 128, 128 * r:1024] = \
                S[j0:13, :, _OFF[r]:_OFF[r] + w]
    # lower blocks are transposes of the stored upper staircase
    for R in range(1, 8):
        for C in range(R):
            out[:, 128 * R:128 * R + 128, 128 * C:128 * C + 128] = \
                out[:, 128 * C:128 * C + 128,
                    128 * R:128 * R + 128].transpose(0, 2, 1)
    return out


# revision 8
# speedup vs baseline: 1.5273x; 1.4703x over previous
"""Trainium2 Bass kernel for nn_AttentionBranch: conv->relu->maxpool->conv->relu
followed by per-location rank-1 Gram outer products (100, 1024, 1024).

Sharding: the 100-location Gram axis is split across 8 NeuronCores
(13/12 locations per core). The conv1 backbone is fully REPLICATED on
every core (bf16) so no collective is needed at all. conv2 is
channel-sliced to each core's needed 136-channel window. The row-major
.view(100, 1024) of the conv2 output is realised through a small bf16
DRAM scratch roundtrip.

Output: per location only the upper "staircase" of 36 of the 64
128x128 blocks of the symmetric Gram matrix is computed and stored
(row-block r keeps columns [128r, 1024)), one contiguous 9216B/partition
store per location; the host reconstructs the lower blocks by
transposition.

Perf notes: PE is pre-warmed with dummy matmuls during the input DMA
so the HAM clock gate is at 2.4GHz when conv1 starts. conv1 runs as
144 N=575 bf16 matmuls (single spatial group, 2-PSUM-bank outputs).
The Gram stage broadcasts each row via a K=2 bf16 matmul, evacuates
PSUM->SBUF bf16 once (split DVE/ACT), then runs the staircase
tensor_scalar ops from SBUF where DVE gets the 4x 16-bit packed mode.
Stores alternate the two HWDGE queues and saturate HBM writes.
"""
import os
import numpy as np

# per-core location starts (each core computes 13 consecutive locations;
# odd cores' first location duplicates the previous core's last)
_LO = [0, 12, 25, 37, 50, 62, 75, 87]
# conv2 channel-slice starts; delta_k = 1024*lo_k - 100*ch_lo_k is 0 (even k)
# or 88 (odd k)
_CH_LO = [0, 122, 256, 378, 512, 634, 768, 890]
_NSL = 136  # channels per conv2 slice (covers 88 + 13*1024 flat elements)
# staircase column offsets: block r (width 1024-128r) starts at _OFF[r]
_OFF = [0, 1024, 1920, 2688, 3328, 3840, 4224, 4480]

_WARMUP = 52     # PE warmup matmuls (keep HAM at 2.4GHz through load phase)
_WARMUP_N = 160

_CACHE = {}


def _build_nc():
    from concourse import bacc, tile, mybir

    f32 = mybir.dt.float32
    f16 = mybir.dt.float16
    bf16 = mybir.dt.bfloat16
    AF = mybir.ActivationFunctionType

    nc = bacc.Bacc("TRN2", target_bir_lowering=False, debug=False)

    inp_d = nc.dram_tensor("inp", [128, 4, 27, 25], bf16, kind="ExternalInput")
    w1_d = nc.dram_tensor("w1t", [128, 4, 9, 512], bf16, kind="ExternalInput")
    b1_d = nc.dram_tensor("b1t", [128, 4], f32, kind="ExternalInput")
    w2_d = nc.dram_tensor("w2t", [128, 4, 9, _NSL], bf16, kind="ExternalInput")
    b2_d = nc.dram_tensor("b2t", [128, 2], f32, kind="ExternalInput")
    selw_d = nc.dram_tensor("selw", [2, 128], bf16, kind="ExternalInput")
    selid_d = nc.dram_tensor("selid", [26, 16], bf16, kind="ExternalInput")
    g_d = nc.dram_tensor("gstair", [13, 128, 4608], f16, kind="ExternalOutput")
    scr_d = nc.dram_tensor("scratch", [137, 100], bf16)

    with tile.TileContext(nc) as tc:
        with tc.tile_pool(name="consts", bufs=1) as cp, \
             tc.tile_pool(name="work", bufs=1) as wp:

            w2sb = cp.tile([128, 4, 9, _NSL], bf16)
            b1sb = cp.tile([128, 4], f32)
            b2sb = cp.tile([128, 2], f32)
            selwsb = cp.tile([2, 128], bf16)
            selidsb = cp.tile([26, 16], bf16)

            convp = tc.alloc_tile_pool(name="convp", bufs=1)
            ps1 = tc.alloc_tile_pool(name="ps1", bufs=1, space="PSUM")
            insb = convp.tile([128, 4, 27, 25], bf16)
            w1sb = [convp.tile([128, 9, 512], bf16, name=f"w1c{c}")
                    for c in range(4)]

            # critical loads first, alternating the two HWDGE queues
            nc.sync.dma_start(out=insb[:], in_=inp_d.ap())
            nc.scalar.dma_start(out=w1sb[0][:], in_=w1_d.ap()[:, 0])
            nc.sync.dma_start(out=w1sb[1][:], in_=w1_d.ap()[:, 1])
            nc.scalar.dma_start(out=w1sb[2][:], in_=w1_d.ap()[:, 2])
            nc.sync.dma_start(out=w1sb[3][:], in_=w1_d.ap()[:, 3])
            nc.scalar.dma_start(out=w2sb[:], in_=w2_d.ap())
            nc.scalar.dma_start(out=b1sb[:], in_=b1_d.ap())
            nc.scalar.dma_start(out=b2sb[:], in_=b2_d.ap())
            nc.scalar.dma_start(out=selwsb[:], in_=selw_d.ap())
            nc.scalar.dma_start(out=selidsb[:], in_=selid_d.ap())

            # conv1 psum: 4 m-chunks x 2 row groups (8 one-bank tiles)
            GB = [(0, 12, 300), (12, 11, 275)]  # (row0, nrows, N)
            c1ps = [[ps1.tile([128, GB[g][2]], f32, name=f"c1ps{m}_{g}")
                     for g in range(2)] for m in range(4)]
            c1sb = wp.tile([128, 4, 24, 24], f32)
            nc.vector.memset(c1sb[:, :, 23:24, :], 0.0)
            nc.vector.memset(c1sb[:, :, :, 23:24], 0.0)

            # PE warmup: junk matmuls with no DMA deps keep the PE busy from
            # preamble end until conv1's inputs land, so HAM is at 8/8
            wamm = wp.tile([128, _WARMUP_N], bf16)
            nc.vector.memset(wamm[:], 0.0)
            for i in range(_WARMUP):
                nc.tensor.matmul(c1ps[0][0][:, 0:_WARMUP_N], wamm[:, 0:128],
                                 wamm[:], start=True, stop=True)

            # ---- conv1 FULL (512 channels), bf16, c-outer so matmuls can
            # start as soon as the first ci-chunk of weights lands ----
            for c in range(4):
                flat_c = insb[:, c].rearrange("p a b -> p (a b)")
                for m in range(4):
                    for t in range(9):
                        dy, dx = t // 3, t % 3
                        for g, (r0, nr, N) in enumerate(GB):
                            s0 = (r0 + dy) * 25 + dx
                            nc.tensor.matmul(
                                c1ps[m][g][:, :],
                                w1sb[c][:, t, 128 * m:128 * m + 128],
                                flat_c[:, s0:s0 + N],
                                start=(c == 0 and t == 0),
                                stop=(c == 3 and t == 8),
                            )

            # relu+bias then maxpool 2x2 ceil per m-chunk -> (512,12,12) bf16
            colmax = wp.tile([128, 4, 24, 12], f32)
            pooled = wp.tile([128, 4, 12, 12], bf16)
            for m in range(4):
                for g, (r0, nr, N) in enumerate(GB):
                    nc.scalar.activation(
                        out=c1sb[:, m, r0:r0 + nr, 0:23],
                        in_=c1ps[m][g][:, :].rearrange(
                            "p (a b) -> p a b", b=25)[:, 0:nr, 0:23],
                        func=AF.Relu,
                        bias=b1sb[:, m:m + 1],
                    )
                cpair = c1sb[:, m].rearrange("p r (w two) -> p r w two", two=2)
                nc.vector.tensor_max(colmax[:, m], cpair[:, :, :, 0],
                                     cpair[:, :, :, 1])
                rpair = colmax[:, m].rearrange("p (r two) w -> p r two w",
                                               two=2)
                nc.vector.tensor_max(pooled[:, m], rpair[:, :, 0, :],
                                     rpair[:, :, 1, :])

            # ---- conv2 slice: 136 output channels, bf16; the small m=1
            # chunk goes FIRST so after the m=0 scratch store every flat
            # load below is immediately ready ----
            ps1.release()
            ps2 = tc.alloc_tile_pool(name="ps2", bufs=2, space="PSUM")
            c2sb = wp.tile([128, 2, 100], bf16)
            for mo, mw, sl in ((128, 8, 1), (0, 128, 0)):
                ps = ps2.tile([128, 100], f32, tag="c2p")
                for c in range(4):
                    for t in range(9):
                        dy, dx = t // 3, t % 3
                        nc.tensor.matmul(
                            ps[0:mw, :],
                            w2sb[:, c, t, mo:mo + mw],
                            pooled[:, c, dy:dy + 10, dx:dx + 10],
                            start=(c == 0 and t == 0),
                            stop=(c == 3 and t == 8),
                        )
                nc.scalar.activation(
                    out=c2sb[0:mw, sl, :],
                    in_=ps[0:mw, :],
                    func=AF.Relu,
                    bias=b2sb[0:mw, sl:sl + 1],
                )
                if sl == 0:
                    nc.sync.dma_start(out=scr_d.ap()[0:128, :],
                                      in_=c2sb[:, 0, :])
                else:
                    nc.scalar.dma_start(out=scr_d.ap()[128:136, :],
                                        in_=c2sb[0:8, 1, :])

            # flat .view(13,1024) rows: T2 holds both delta variants for the
            # column factors; t2row holds them as 2 long rows for the PE
            # row-broadcast.
            flat = scr_d.ap().rearrange("a b -> (a b)")
            T2 = wp.tile([26, 1024], bf16)
            t2row = wp.tile([2, 13312], bf16)
            nc.sync.dma_start(out=t2row[0:1, :], in_=flat[0:13312])
            nc.scalar.dma_start(out=t2row[1:2, :], in_=flat[88:13400])
            nc.sync.dma_start(
                out=T2[0:13, :],
                in_=flat[0:13312].rearrange("(p i) -> p i", i=1024))
            nc.scalar.dma_start(
                out=T2[13:26, :],
                in_=flat[88:13400].rearrange("(p i) -> p i", i=1024))

            ps2.release()
            convp.release()

            # column factors: ccol[p, r, l] = v_l[128r + p], selecting the
            # delta-0/delta-88 variant via the selid block-diagonal
            psC = tc.alloc_tile_pool(name="psC", bufs=1, space="PSUM")
            pc = psC.tile([128, 8, 16], f32)
            ccol = wp.tile([128, 8, 16], f32)
            for r in range(8):
                nc.tensor.matmul(pc[:, r, 0:13], T2[:, 128 * r:128 * r + 128],
                                 selidsb[0:26, 0:13], start=True, stop=True)
            nc.vector.tensor_copy(ccol[:, :, 0:13], pc[:, :, 0:13])

            psB = tc.alloc_tile_pool(name="psB", bufs=3, space="PSUM")
            vp = tc.alloc_tile_pool(name="bcast", bufs=3)
            sp = tc.alloc_tile_pool(name="stage", bufs=5)

            # ---- Gram staircase ----
            # ACT takes widths {640, 256, 128}; DVE (4x packed mode) takes
            # {1024, 896, 768, 512, 384}
            ACT_R = (3, 6, 7)
            for li in range(13):
                bp = psB.tile([128, 1024], f32, tag="bc")
                nc.tensor.matmul(bp[:, 0:512], selwsb[:],
                                 t2row[:, 1024 * li:1024 * li + 512],
                                 start=True, stop=True)
                nc.tensor.matmul(bp[:, 512:1024], selwsb[:],
                                 t2row[:, 1024 * li + 512:1024 * (li + 1)],
                                 start=True, stop=True)
                bc = vp.tile([128, 1024], bf16, tag="bcs")
                nc.vector.tensor_copy(bc[:, 0:512], bp[:, 0:512])
                nc.scalar.activation(bc[:, 512:1024], bp[:, 512:1024],
                                     func=AF.Copy)
                st = sp.tile([128, 4608], f16, tag="st")
                for r in range(8):
                    w = 1024 - 128 * r
                    src = bc[:, 128 * r:1024]
                    dst = st[:, _OFF[r]:_OFF[r] + w]
                    col = ccol[:, r, li:li + 1]
                    if r in ACT_R:
                        nc.scalar.activation(dst, src, func=AF.Copy,
                                             scale=col)
                    else:
                        nc.vector.tensor_scalar_mul(dst, src, col)
                eng = nc.sync if li % 2 == 0 else nc.scalar
                eng.dma_start(out=g_d.ap()[li], in_=st[:])
            psB.release()
            psC.release()
            sp.release()
            vp.release()

    nc.compile()
    return nc


def _get_nc():
    if "nc" not in _CACHE:
        _CACHE["nc"] = _build_nc()
    return _CACHE["nc"]


def _host_prep(input, w1, b1, w2, b2):
    import ml_dtypes
    bf = ml_dtypes.bfloat16

    x = np.asarray(input, np.float32).reshape(512, 25, 25)
    w1 = np.asarray(w1, np.float32)
    w2 = np.asarray(w2, np.float32)
    b1 = np.asarray(b1, np.float32)
    b2 = np.asarray(b2, np.float32)

    inp = np.zeros((4, 128, 27, 25), np.float32)
    inp[:, :, :25, :] = x.reshape(4, 128, 25, 25)
    inp = np.ascontiguousarray(inp.transpose(1, 0, 2, 3)).astype(bf)

    w1t = w1.reshape(512, 512, 9).transpose(1, 2, 0)          # [ci, 9, co]
    w1t = np.ascontiguousarray(
        w1t.reshape(4, 128, 9, 512).transpose(1, 0, 2, 3)).astype(bf)
    b1t = np.ascontiguousarray(b1.reshape(4, 128).T)          # [128, 4]

    common = {"inp": inp, "w1t": w1t, "b1t": b1t}
    in_maps = []
    for k in range(8):
        ch = _CH_LO[k]
        nval = min(1024, ch + _NSL) - ch
        wsl = np.zeros((_NSL, 512, 9), np.float32)
        wsl[:nval] = w2.reshape(1024, 512, 9)[ch:ch + nval]
        w2t = wsl.transpose(1, 2, 0)                           # [512,9,136]
        w2t = np.ascontiguousarray(
            w2t.reshape(4, 128, 9, _NSL).transpose(1, 0, 2, 3)).astype(bf)
        bsl = np.zeros(256, np.float32)
        bsl[:nval] = b2[ch:ch + nval]
        b2t = np.ascontiguousarray(bsl.reshape(2, 128).T)
        s0 = 1.0 if (1024 * _LO[k] - 100 * ch) == 0 else 0.0
        selw = np.zeros((2, 128), np.float32)
        selw[0, :] = s0
        selw[1, :] = 1.0 - s0
        selid = np.zeros((26, 16), np.float32)
        selid[0:13, 0:13] = s0 * np.eye(13, dtype=np.float32)
        selid[13:26, 0:13] = (1.0 - s0) * np.eye(13, dtype=np.float32)
        in_maps.append({**common, "w2t": w2t, "b2t": b2t,
                        "selw": selw.astype(bf), "selid": selid.astype(bf)})
    return in_maps


def kernel(input, w1, b1, w2, b2):
    from concourse import bass_utils

    nc = _get_nc()
    in_maps = _host_prep(input, w1, b1, w2, b2)

    prof_dir = os.environ.get("GRAM_KERNEL_PROFILE_DIR")
    if prof_dir:
        from trn_agent_boot.trn_boot import _ntff_profile_via_ctypes
        hook = _ntff_profile_via_ctypes('/opt/axon/libaxon_pjrt.so')
        with hook(prof_dir, [0]):
            res = bass_utils.run_bass_kernel_spmd(
                nc, in_maps, core_ids=list(range(8)))
    else:
        res = bass_utils.run_bass_kernel_spmd(
            nc, in_maps, core_ids=list(range(8)))

    out = np.empty((100, 1024, 1024), np.float32)
    for k in range(8):
        S = np.asarray(res.results[k]["gstair"])   # [13, 128, 4608] f16
        j0 = k % 2   # odd cores' first row duplicates previous core's last
        lo = _LO[k]
        for r in range(8):
            w = 1024 - 128 * r
            out[lo + j0:lo + 13, 128 * r:128 * r + 128, 128 * r:1024] = \
                S[j0:13, :, _OFF[r]:_OFF[r] + w]
    # lower blocks are transposes of the stored upper staircase
    for R in range(1, 8):
        for C in range(R):
            out[:, 128 * R:128 * R + 128, 128 * C:128 * C + 128] = \
                out[:, 128 * C:128 * C + 128,
                    128 * R:128 * R + 128].transpose(0, 2, 1)
    return out
